# revision 32
# baseline (speedup 1.0000x reference)
"""Trainium2 Bass kernel for Kaldi LinearResample (16 kHz -> 22.05 kHz) on [8, 960000].

out[b, 441*(4q+m) + p] = sum_i x[b, 1280*q + 320*m - 6 + i] * B[i, p] with B the
[384, 441] polyphase tap matrix (13 taps/phase).  Pure data parallel: one batch
row per core.

The HOST pre-transposes the input into a deduplicated u-major layout (a strided
numpy copy + bf16 cast), so the device does NO transposes at all:

  XT[u, 1281*j + t] = xpad6[128*(1280*j + t) + u]     (bf16, t in [0, 1281))

per q-tile j a [128, 1281] block whose partition dim is the sample offset u
within each 128-sample chunk; the per-chunk matmul lhsT is the stride-10 slice
XT[:, j, c : c+1271 : 10].

Schedule (from NTFF trace analysis of the 29.4us baseline; now ~27.7us):
 - bfilt rides FIRST on the scalar ring while input tile 0 rides first on
   the sync ring: the rings feed ~11-20ns/descriptor at startup (128 descs
   per DMA), so these two must go in parallel; both land ~11.3us (trace)
   and the first real matmul follows immediately.  Input tiles alternate
   rings (even sync / odd scalar) and all input triggers are issued before
   the tile loop so a store's semaphore wait can never stall them.
 - Per tile: 14 accumulating bf16 matmuls against 7 shifted filter tiles
   (packed to their structural column ranges, 965 cols) into two 2-bank
   f32 PSUM halves (phases 0,1 / 2,3; 4-deep rotation over the 8 banks)
   -> ACT copies phases 0,1 in parallel with DVE phases 2,3 into ONE
   shared [128, 4, 441] bf16 tile -> ONE store per tile (3528B/partition
   descriptors, rings alternate).  Output DRAM layout [q, m, p] is exactly
   the final layout: the host only reshapes.
 - NO gpsimd/SWDGE DMA anywhere: a software-DGE store path costs a 5.2us
   gpsimd dge-drain right before the exit barrier.
 - Output padded q rows are trimmed at the store (tile 5 stores 110 rows).
 - PE pre-warm: HAM holds the PE clock low until it has seen a ~3.4us
   window of high activity; 7 N=512 dummy matmuls burn that window while
   the first input DMA flies, and 2 narrow bridge warms keep the window
   alive if the input lands late.  Output tile pool bufs=6 so a store
   completion WAR can never stall a copy.
 - PSUM accumulation semantics (hw-verified): start=True flips the
   accumulation epoch of the bank holding the instruction's base address;
   within the current epoch, a write to an entry with a stale tag
   REPLACES, a write to a current-tagged entry ACCUMULATES.  That is why
   the per-m chunk matmuls may overlap column ranges with start only on
   the first chunk.
"""

import math

import numpy as np

N_IN = 960000
P_PH = 441
NQ = 750            # real q rows (4 blocks of 320 samples each)
NTILE = 6
TCOLS = 1281        # u-major columns per q-tile (1280 + 1 shared boundary col)
XPAD = 983168       # 128*7680 + 127 + 1
N_OUT = NQ * 4 * P_PH
N_CORES = 8

# per block-phase m: (chunk c, filter-tile shift off = 128c - 320m)
USE = {0: [(0, 0), (1, 128), (2, 256)],
       1: [(2, -64), (3, 64), (4, 192), (5, 320)],
       2: [(5, 0), (6, 128), (7, 256)],
       3: [(7, -64), (8, 64), (9, 192), (10, 320)]}
# packed B column order
PACK = [0, 128, 256, -64, 64, 192, 320]

_ORIG, _NEW, _LPW = 16000, 22050, 6


def _tables():
    """Packed filter [128, sum(widths)] bf16-able f32 + per-shift col ranges."""
    base = math.gcd(_ORIG, _NEW)
    P = _NEW // base
    cutoff = 0.99 * 0.5 * min(_ORIG, _NEW)
    ww = _LPW / (2.0 * cutoff)
    out_t = np.arange(P, dtype=np.float64) / _NEW
    min_i = np.ceil((out_t - ww) * _ORIG)
    max_i = np.floor((out_t + ww) * _ORIG)
    W = int((max_i - min_i + 1).max())
    j = np.arange(W, dtype=np.float64)
    inp_i = min_i[:, None] + j[None, :]
    dt = inp_i / _ORIG - out_t[:, None]
    w = np.zeros_like(dt)
    inside = np.abs(dt) < ww
    w[inside] = 0.5 * (1.0 + np.cos(2.0 * np.pi * cutoff / _LPW * dt[inside]))
    zero = dt == 0.0
    nz = ~zero
    w[nz] *= np.sin(2.0 * np.pi * cutoff * dt[nz]) / (np.pi * dt[nz])
    w[zero] *= 2.0 * cutoff
    w /= _ORIG
    fi = min_i.astype(np.int64)
    wf = w.astype(np.float32)
    Bfull = np.zeros((384, P), dtype=np.float32)
    for p in range(P):
        for jj in range(W):
            Bfull[fi[p] + 6 + jj, p] += wf[p, jj]
    lo = fi + 6
    colr, boff, packed = {}, {}, []
    pos = 0
    for off in PACK:
        cols = np.where((lo + W - 1 >= off) & (lo <= off + 127))[0]
        c0, c1 = int(cols.min()), int(cols.max()) + 1
        colr[off] = (c0, c1)
        boff[off] = pos
        t = np.zeros((128, c1 - c0), dtype=np.float32)
        for r in range(128):
            src = off + r
            if 0 <= src < 384:
                t[r] = Bfull[src, c0:c1]
        packed.append(t)
        pos += c1 - c0
    return np.concatenate(packed, axis=1), colr, boff


_COLR: dict = {}
_BOFF: dict = {}
_BW = 0
_CACHE: dict = {}


def _build():
    if "nc" in _CACHE:
        return _CACHE["nc"]

    import concourse.bass as bass
    import concourse.tile as tile
    from concourse import bacc, mybir

    F32 = mybir.dt.float32
    BF16 = mybir.dt.bfloat16

    bw = _BW

    nc = bacc.Bacc("TRN2", target_bir_lowering=False, debug=False,
                   num_devices=N_CORES)
    x_dram = nc.declare_dram_parameter("xt", [128 * NTILE * TCOLS], BF16,
                                       isOutput=False)
    b_dram = nc.declare_dram_parameter("bfilt", [128, bw], BF16, isOutput=False)
    o_dram = nc.declare_dram_parameter("out", [NQ * 4 * P_PH], BF16,
                                       isOutput=True)
    xh = x_dram.ap().tensor
    oh = o_dram.ap().tensor
    NCOL = NTILE * TCOLS

    with tile.TileContext(nc) as tc:
        with (
            tc.tile_pool(name="sb", bufs=1) as spool,
            tc.tile_pool(name="pacc", bufs=2, space="PSUM") as paccpool,
        ):
            scratch = spool.tile([128, 512], BF16)
            nc.vector.memset(scratch[:], 0.0)
            warmsb = spool.tile([128, 128], BF16)
            nc.scalar.mul(warmsb[:], scratch[:, 0:128], 1.0)

            # bfilt FIRST on the scalar ring (in parallel with tile 0 on the
            # sync ring): the ring feed-paces ~19ns/descriptor at startup,
            # so serializing bfilt ahead of tile 0 on ONE ring would delay
            # the first real matmul by ~2.4us.  Both land ~11.3us (trace).
            bsb = spool.tile([128, bw], BF16)
            nc.scalar.dma_start(bsb[:], b_dram[:, :])
            xtall = spool.tile([128, NTILE, TCOLS], BF16)
            for j in range(NTILE):
                eng = nc.sync if j % 2 == 0 else nc.scalar
                eng.dma_start(
                    xtall[:, j, :],
                    bass.AP(xh, TCOLS * j, [[NCOL, 128], [1, TCOLS]]),
                )

            # PE pre-warm while the first input DMA is in flight: HAM holds
            # the PE clock at half speed until it has seen a ~3.4us window
            # of high activity; 7 N=512 dummies provide exactly that.
            warm = paccpool.tile([128, 2, 512], F32, name="warm", tag="pa")
            for i in range(7):
                nc.tensor.matmul(warm[:, 0, :], scratch[:, 0:128],
                                 scratch[:], start=True, stop=True,
                                 skip_group_check=True)
            # two narrow bridge warms: if the input lands late, these keep
            # the PE activity window alive so HAM doesn't drop the clock
            # right as the real stream begins
            for i in range(2):
                nc.tensor.matmul(warm[:, 1, 0:256], scratch[:, 0:128],
                                 scratch[:, 0:256], start=True, stop=True,
                                 skip_group_check=True)

            W4 = 4 * P_PH
            ot = None
            for j in range(NTILE):
                pacca = paccpool.tile([128, 2, 512], F32, name=f"pa{j}",
                                      tag="pa")
                paccb = paccpool.tile([128, 2, 512], F32, name=f"pb{j}",
                                      tag="pb")
                for m in range(4):
                    uses = USE[m]
                    pacc = pacca if m < 2 else paccb
                    for ui, (c, off) in enumerate(uses):
                        c0, c1 = _COLR[off]
                        nc.tensor.matmul(
                            pacc[:, m % 2, c0:c1],
                            xtall[:, j, c:c + 1271:10],
                            bsb[:, _BOFF[off]:_BOFF[off] + (c1 - c0)],
                            start=(ui == 0),
                            stop=(ui == len(uses) - 1),
                        )

                # ACT copies phases 0,1 in parallel with DVE phases 2,3
                # into ONE shared output tile -> a single 3528B/partition
                # store per tile.  (GPSIMD cannot access PSUM.)
                ot = spool.tile([128, 4, P_PH], BF16, name=f"ot{j}",
                                bufs=6, tag="ot")
                nc.scalar.mul(ot[:, 0:2, :], pacca[:, :, 0:P_PH], 1.0)
                nc.vector.tensor_copy(ot[:, 2:4, :], paccb[:, :, 0:P_PH])

                rows = NQ - 128 * j if j == NTILE - 1 else 128
                eng = nc.sync if j % 2 == 0 else nc.scalar
                eng.dma_start(
                    bass.AP(oh, W4 * 128 * j, [[W4, rows], [1, W4]]),
                    ot[0:rows, :, :],
                )

    nc.compile()
    _CACHE["nc"] = nc
    return nc


def _prep():
    import ml_dtypes

    if "bmat" not in _CACHE:
        global _BW
        bmat, colr, boff = _tables()
        _COLR.update(colr)
        _BOFF.update(boff)
        _BW = bmat.shape[1]
        _CACHE["bmat"] = bmat.astype(ml_dtypes.bfloat16)
    return _CACHE["bmat"]


def _make_xt(x: np.ndarray) -> np.ndarray:
    """[128*6*1281] bf16 u-major: XT[u, 1281j + t] = xpad6[128*(1280j + t) + u]."""
    import ml_dtypes

    xpad = np.zeros(XPAD, dtype=ml_dtypes.bfloat16)
    xpad[6:6 + N_IN] = x.astype(ml_dtypes.bfloat16)
    v = np.lib.stride_tricks.as_strided(
        xpad, shape=(128, NTILE, TCOLS),
        strides=(2, 2 * 1280 * 128, 2 * 128))
    return np.ascontiguousarray(v).reshape(-1)


def _run(waveforms: np.ndarray, trace: bool = False):
    from concourse.bass_utils import run_bass_kernel_spmd

    bmat = _prep()
    nc = _build()
    in_maps = [
        {"xt": _make_xt(np.ascontiguousarray(waveforms[b], dtype=np.float32)),
         "bfilt": bmat}
        for b in range(N_CORES)
    ]
    last_err = None
    for attempt in range(3):
        try:
            res = run_bass_kernel_spmd(nc, in_maps, list(range(N_CORES)),
                                       trace=trace)
            out = np.stack([
                np.asarray(res.results[b]["out"]).reshape(N_OUT)
                for b in range(N_CORES)
            ]).astype(np.float32)
            if not np.isfinite(out).all():
                raise RuntimeError("non-finite output (transient device "
                                   "corruption); retrying")
            return out, res
        except Exception as e:  # transient NRT device faults recover on retry
            last_err = e
            import time
            time.sleep(10)
    raise last_err


def kernel(waveforms: np.ndarray) -> np.ndarray:
    out, _ = _run(np.asarray(waveforms))
    return out


# revision 33
# speedup vs baseline: 1.0224x; 1.0224x over previous
"""Trainium2 Bass kernel for Kaldi LinearResample (16 kHz -> 22.05 kHz) on [8, 960000].

out[b, 441*(4q+m) + p] = sum_i x[b, 1280*q + 320*m - 6 + i] * B[i, p] with B the
[384, 441] polyphase tap matrix (13 taps/phase).  Pure data parallel: one batch
row per core.

The HOST pre-transposes the input into a deduplicated u-major layout (a strided
numpy copy + bf16 cast), so the device does NO transposes at all:

  XT[u, 1281*j + t] = xpad6[128*(1280*j + t) + u]     (bf16, t in [0, 1281))

per q-tile j a [128, 1281] block whose partition dim is the sample offset u
within each 128-sample chunk; the per-chunk matmul lhsT is the stride-10 slice
XT[:, j, c : c+1271 : 10].

Schedule (from NTFF trace analysis of the 29.4us baseline; now ~27.7us):
 - bfilt rides FIRST on the scalar ring while input tile 0 rides first on
   the sync ring: the rings feed ~11-20ns/descriptor at startup (128 descs
   per DMA), so these two must go in parallel; both land ~11.3us (trace)
   and the first real matmul follows immediately.  Input tiles alternate
   rings (even sync / odd scalar) and all input triggers are issued before
   the tile loop so a store's semaphore wait can never stall them.
 - Per tile: 14 accumulating bf16 matmuls against 7 shifted filter tiles
   (packed to their structural column ranges, 965 cols) into two 2-bank
   f32 PSUM halves (phases 0,1 / 2,3; 4-deep rotation over the 8 banks)
   -> ACT copies phases 0,1 in parallel with DVE phases 2,3 into ONE
   shared [128, 4, 441] bf16 tile -> ONE store per tile (3528B/partition
   descriptors, rings alternate).  Output DRAM layout [q, m, p] is exactly
   the final layout: the host only reshapes.
 - NO gpsimd/SWDGE DMA anywhere: a software-DGE store path costs a 5.2us
   gpsimd dge-drain right before the exit barrier.
 - Output padded q rows are trimmed at the store (tile 5 stores 110 rows).
 - PE pre-warm: HAM holds the PE clock low until it has seen a ~3.4us
   window of high activity; 7 N=512 dummy matmuls burn that window while
   the first input DMA flies, and 2 narrow bridge warms keep the window
   alive if the input lands late.  Output tile pool bufs=6 so a store
   completion WAR can never stall a copy.
 - PSUM accumulation semantics (hw-verified): start=True flips the
   accumulation epoch of the bank holding the instruction's base address;
   within the current epoch, a write to an entry with a stale tag
   REPLACES, a write to a current-tagged entry ACCUMULATES.  That is why
   the per-m chunk matmuls may overlap column ranges with start only on
   the first chunk.
"""

import math

import numpy as np

N_IN = 960000
P_PH = 441
NQ = 750            # real q rows (4 blocks of 320 samples each)
NTILE = 6
TCOLS = 1281        # u-major columns per q-tile (1280 + 1 shared boundary col)
XPAD = 983168       # 128*7680 + 127 + 1
N_OUT = NQ * 4 * P_PH
N_CORES = 8

# per block-phase m: (chunk c, filter-tile shift off = 128c - 320m)
USE = {0: [(0, 0), (1, 128), (2, 256)],
       1: [(2, -64), (3, 64), (4, 192), (5, 320)],
       2: [(5, 0), (6, 128), (7, 256)],
       3: [(7, -64), (8, 64), (9, 192), (10, 320)]}
# packed B column order
PACK = [0, 128, 256, -64, 64, 192, 320]

_ORIG, _NEW, _LPW = 16000, 22050, 6


def _tables():
    """Packed filter [128, sum(widths)] bf16-able f32 + per-shift col ranges."""
    base = math.gcd(_ORIG, _NEW)
    P = _NEW // base
    cutoff = 0.99 * 0.5 * min(_ORIG, _NEW)
    ww = _LPW / (2.0 * cutoff)
    out_t = np.arange(P, dtype=np.float64) / _NEW
    min_i = np.ceil((out_t - ww) * _ORIG)
    max_i = np.floor((out_t + ww) * _ORIG)
    W = int((max_i - min_i + 1).max())
    j = np.arange(W, dtype=np.float64)
    inp_i = min_i[:, None] + j[None, :]
    dt = inp_i / _ORIG - out_t[:, None]
    w = np.zeros_like(dt)
    inside = np.abs(dt) < ww
    w[inside] = 0.5 * (1.0 + np.cos(2.0 * np.pi * cutoff / _LPW * dt[inside]))
    zero = dt == 0.0
    nz = ~zero
    w[nz] *= np.sin(2.0 * np.pi * cutoff * dt[nz]) / (np.pi * dt[nz])
    w[zero] *= 2.0 * cutoff
    w /= _ORIG
    fi = min_i.astype(np.int64)
    wf = w.astype(np.float32)
    Bfull = np.zeros((384, P), dtype=np.float32)
    for p in range(P):
        for jj in range(W):
            Bfull[fi[p] + 6 + jj, p] += wf[p, jj]
    lo = fi + 6
    colr, boff, packed = {}, {}, []
    pos = 0
    for off in PACK:
        cols = np.where((lo + W - 1 >= off) & (lo <= off + 127))[0]
        c0, c1 = int(cols.min()), int(cols.max()) + 1
        colr[off] = (c0, c1)
        boff[off] = pos
        t = np.zeros((128, c1 - c0), dtype=np.float32)
        for r in range(128):
            src = off + r
            if 0 <= src < 384:
                t[r] = Bfull[src, c0:c1]
        packed.append(t)
        pos += c1 - c0
    return np.concatenate(packed, axis=1), colr, boff


_COLR: dict = {}
_BOFF: dict = {}
_BW = 0
_CACHE: dict = {}


def _build():
    if "nc" in _CACHE:
        return _CACHE["nc"]

    import concourse.bass as bass
    import concourse.tile as tile
    from concourse import bacc, mybir

    F32 = mybir.dt.float32
    BF16 = mybir.dt.bfloat16

    bw = _BW

    nc = bacc.Bacc("TRN2", target_bir_lowering=False, debug=False,
                   num_devices=N_CORES)
    x_dram = nc.declare_dram_parameter("xt", [128 * NTILE * TCOLS], BF16,
                                       isOutput=False)
    b_dram = nc.declare_dram_parameter("bfilt", [128, bw], BF16, isOutput=False)
    o_dram = nc.declare_dram_parameter("out", [NQ * 4 * P_PH], BF16,
                                       isOutput=True)
    xh = x_dram.ap().tensor
    oh = o_dram.ap().tensor
    NCOL = NTILE * TCOLS

    with tile.TileContext(nc) as tc:
        with (
            tc.tile_pool(name="sb", bufs=1) as spool,
            tc.tile_pool(name="pacc", bufs=2, space="PSUM") as paccpool,
        ):
            scratch = spool.tile([128, 512], BF16)
            nc.vector.memset(scratch[:], 0.0)
            warmsb = spool.tile([128, 128], BF16)
            nc.scalar.mul(warmsb[:], scratch[:, 0:128], 1.0)

            # bfilt FIRST on the scalar ring (in parallel with tile 0 on the
            # sync ring): the ring feed-paces ~19ns/descriptor at startup,
            # so serializing bfilt ahead of tile 0 on ONE ring would delay
            # the first real matmul by ~2.4us.  Both land ~11.3us (trace).
            bsb = spool.tile([128, bw], BF16)
            nc.scalar.dma_start(bsb[:], b_dram[:, :])
            xtall = spool.tile([128, NTILE, TCOLS], BF16)
            for j in range(NTILE):
                eng = nc.sync if j % 2 == 0 else nc.scalar
                eng.dma_start(
                    xtall[:, j, :],
                    bass.AP(xh, TCOLS * j, [[NCOL, 128], [1, TCOLS]]),
                )

            # PE pre-warm while the first input DMA is in flight: HAM holds
            # the PE clock at half speed until it has seen a ~3.4us window
            # of high activity; 7 N=512 dummies provide exactly that.
            warm = paccpool.tile([128, 2, 512], F32, name="warm", tag="pa")
            for i in range(7):
                nc.tensor.matmul(warm[:, 0, :], scratch[:, 0:128],
                                 scratch[:], start=True, stop=True,
                                 skip_group_check=True)
            # two narrow bridge warms: if the input lands late, these keep
            # the PE activity window alive so HAM doesn't drop the clock
            # right as the real stream begins
            for i in range(2):
                nc.tensor.matmul(warm[:, 1, 0:256], scratch[:, 0:128],
                                 scratch[:, 0:256], start=True, stop=True,
                                 skip_group_check=True)

            W4 = 4 * P_PH
            ot = None
            for j in range(NTILE):
                pacca = paccpool.tile([128, 2, 512], F32, name=f"pa{j}",
                                      tag="pa")
                paccb = paccpool.tile([128, 2, 512], F32, name=f"pb{j}",
                                      tag="pb")
                for m in range(4):
                    uses = USE[m]
                    pacc = pacca if m < 2 else paccb
                    for ui, (c, off) in enumerate(uses):
                        c0, c1 = _COLR[off]
                        nc.tensor.matmul(
                            pacc[:, m % 2, c0:c1],
                            xtall[:, j, c:c + 1271:10],
                            bsb[:, _BOFF[off]:_BOFF[off] + (c1 - c0)],
                            start=(ui == 0),
                            stop=(ui == len(uses) - 1),
                        )

                # ACT copies phases 0,1 in parallel with DVE phases 2,3
                # into ONE shared output tile -> a single 3528B/partition
                # store per tile.  (GPSIMD cannot access PSUM.)  For the
                # last two tiles the copies split into per-phase quarters:
                # each quarter's dependency is only ITS m-group's matmuls,
                # so phases 0-2 copy while later m-groups still run and
                # only one 441-col copy (~0.65us) remains after the final
                # matmul, instead of a full 882-col half.
                ot = spool.tile([128, 4, P_PH], BF16, name=f"ot{j}",
                                bufs=6, tag="ot")
                if j < NTILE - 2:
                    nc.scalar.mul(ot[:, 0:2, :], pacca[:, :, 0:P_PH], 1.0)
                    nc.vector.tensor_copy(ot[:, 2:4, :], paccb[:, :, 0:P_PH])
                else:
                    nc.scalar.mul(ot[:, 0, :], pacca[:, 0, 0:P_PH], 1.0)
                    nc.scalar.mul(ot[:, 1, :], pacca[:, 1, 0:P_PH], 1.0)
                    nc.vector.tensor_copy(ot[:, 2, :], paccb[:, 0, 0:P_PH])
                    nc.vector.tensor_copy(ot[:, 3, :], paccb[:, 1, 0:P_PH])

                rows = NQ - 128 * j if j == NTILE - 1 else 128
                eng = nc.sync if j % 2 == 0 else nc.scalar
                eng.dma_start(
                    bass.AP(oh, W4 * 128 * j, [[W4, rows], [1, W4]]),
                    ot[0:rows, :, :],
                )

    nc.compile()
    _CACHE["nc"] = nc
    return nc


def _prep():
    import ml_dtypes

    if "bmat" not in _CACHE:
        global _BW
        bmat, colr, boff = _tables()
        _COLR.update(colr)
        _BOFF.update(boff)
        _BW = bmat.shape[1]
        _CACHE["bmat"] = bmat.astype(ml_dtypes.bfloat16)
    return _CACHE["bmat"]


def _make_xt(x: np.ndarray) -> np.ndarray:
    """[128*6*1281] bf16 u-major: XT[u, 1281j + t] = xpad6[128*(1280j + t) + u]."""
    import ml_dtypes

    xpad = np.zeros(XPAD, dtype=ml_dtypes.bfloat16)
    xpad[6:6 + N_IN] = x.astype(ml_dtypes.bfloat16)
    v = np.lib.stride_tricks.as_strided(
        xpad, shape=(128, NTILE, TCOLS),
        strides=(2, 2 * 1280 * 128, 2 * 128))
    return np.ascontiguousarray(v).reshape(-1)


def _run(waveforms: np.ndarray, trace: bool = False):
    from concourse.bass_utils import run_bass_kernel_spmd

    bmat = _prep()
    nc = _build()
    in_maps = [
        {"xt": _make_xt(np.ascontiguousarray(waveforms[b], dtype=np.float32)),
         "bfilt": bmat}
        for b in range(N_CORES)
    ]
    last_err = None
    for attempt in range(3):
        try:
            res = run_bass_kernel_spmd(nc, in_maps, list(range(N_CORES)),
                                       trace=trace)
            out = np.stack([
                np.asarray(res.results[b]["out"]).reshape(N_OUT)
                for b in range(N_CORES)
            ]).astype(np.float32)
            if not np.isfinite(out).all():
                raise RuntimeError("non-finite output (transient device "
                                   "corruption); retrying")
            return out, res
        except Exception as e:  # transient NRT device faults recover on retry
            last_err = e
            import time
            time.sleep(10)
    raise last_err


def kernel(waveforms: np.ndarray) -> np.ndarray:
    out, _ = _run(np.asarray(waveforms))
    return out
